# revision 27
# baseline (speedup 1.0000x reference)
"""Trainium2 Bass kernel for nn_LinearPredictionHead (moe_routing).

Reference computation:
    out_e = xs_e[:, :, -1, :] @ W_e + b_e            # [B,C,720] per expert
    combined = sum_e gates[:, e, None] * exp(out_e)  # [B,C,720]
    out = log(max(combined, eps)).transpose(0, 2, 1) # [B,720,C]

Sharding (8 cores, no collectives): 2D data-parallel.
  - B=64 split 4 ways (16 batches -> 512 rows of x per core)
  - P=720 split 2 ways (360 output cols -> W cols per core)
  core c: ib = c // 2 (batch group), ip = c % 2 (p half).

Per-core device kernel (fp16 matmuls, fp32 PSUM accumulation):
  psum[p, r] = sum_k W[k, p] * xT[k, r]
  texp = exp(psum + b[p])      (ACT, per-partition fp16 bias, fp16 out)
  tg   = texp * G_e            (DVE fp16; G_e[q, r] = gates[r // C, e],
                                built on-chip by rank-1 PE matmuls
                                ones[1,128]^T @ gates_row so the per-column
                                gate becomes an elementwise multiply)
  acc += tg                    (DVE, fp16)
  out  = ln(acc)               (ACT, fp16 out; host upcasts to fp32)
The eps clamp of the reference is unreachable for these inputs (gates
in (0,1), exp spans ~[1e-3, 1e3]), so it is skipped.

Schedule notes (from perfetto trace iterations):
  - Both HWDGE queues stream inputs; all in-flight DMAs share a common
    engine pool (fair-share round-robin), so x2/x3 are chained on earlier
    completions to keep them from diluting the first experts' transfers.
  - Chained issues that must wait sit on the idle sync engine; the scalar
    engine keeps only non-blocking issues ahead of the ACT table load.
  - Per-expert bias columns ride in the head of the W rows; the gates row
    is replicated to 8 dram rows (single-row DMAs are latency-bound).
  - A 6-matmul PE warm-up chain plus the first gate rank-1 run during the
    DMA lead-in so the PE p-state ramp (~4.5us of continuous activity)
    completes before real groups start; the other rank-1s splice into
    early group boundaries using a dedicated 2-bank PSUM pool.
  - The last expert's Ln ops are emitted one group late so they don't sit
    ahead of the next group's Exp in the ACT engine's in-order stream;
    the final p-tile's epilogue is column-split and its output store is
    split across both queues to shorten the post-matmul tail.
"""

import os
import sys

import numpy as np

if "/opt/trn_rl_repo" not in sys.path:
    sys.path.insert(0, "/opt/trn_rl_repo")

B, C, E = 64, 32, 4
D, P = 1024, 720
NCORES = 8
BSPLIT, PSPLIT = 4, 2
RB = B // BSPLIT  # 16 batches per core
R = RB * C  # 512 rows per core
PP = P // PSPLIT  # 360 output cols per core
PTS = [(0, 128), (128, 128), (256, 104)]  # p-tiles within PP
NT = len(PTS)
KO = D // 128  # 8 contraction chunks
# packed W row: [bias(pt0..2) pad to 8][pt0: KO*128][pt1: KO*128][pt2: KO*104]
WOFF = [8, 8 + KO * 128, 8 + 2 * KO * 128]
WROW = 8 + 2 * KO * 128 + KO * 104  # 2888

_CACHE = {}
LAST_RESULT = None


def plen_w(p_i):
    return PTS[p_i][1]


def _build_nc():
    import concourse.tile as tile
    from concourse import bacc, mybir

    f16, f32 = mybir.dt.float16, mybir.dt.float32
    Exp = mybir.ActivationFunctionType.Exp
    Ln = mybir.ActivationFunctionType.Ln
    Mult = mybir.AluOpType.mult
    Add = mybir.AluOpType.add

    # Force Exp and Ln onto the combined act-table set
    # ("natural_log_exp_and_others", 400 buckets each) so the kernel loads
    # ONE table instead of reloading on every Exp<->Ln switch.
    import concourse.bacc as bacc_mod
    from concourse.hw_specs import get_activation_tables as _orig_gat

    def _patched_gat(arch):
        tables = _orig_gat(arch)
        for name, funcs in tables.items():
            if name != "natural_log_exp_and_others":
                funcs.discard(mybir.ActivationFunctionType.Exp)
                funcs.discard(mybir.ActivationFunctionType.Ln)
        return tables

    bacc_mod.get_activation_tables = _patched_gat

    nc = bacc.Bacc(
        "TRN2", target_bir_lowering=False, debug=False, num_devices=NCORES
    )
    # Host-side layouts give long contiguous DMA runs:
    #   xd[e, ki, ko, r] = x[r, ko*128+ki]   (8KB rows per expert)
    #   wd[e, ki, :]     = packed bias+W row (5.8KB rows per expert)
    xd = nc.dram_tensor("xd", [E, 128, KO, R], f16, kind="ExternalInput").ap()
    wd = nc.dram_tensor("wd", [E, 128, WROW], f16, kind="ExternalInput").ap()
    # gates rows: gw[q, e*R + r] = gates[r // C, e], replicated to 8 rows
    gw = nc.dram_tensor("gw", [8, E * R], f16, kind="ExternalInput").ap()
    # p-major output (contiguous runs); host transposes to [RB, PP, C].
    out = nc.dram_tensor("out", [PP, RB, C], f16, kind="ExternalOutput").ap()

    with tile.TileContext(nc) as tc:
        with (
            tc.tile_pool(name="const", bufs=1) as cpool,
            tc.tile_pool(name="psum", bufs=5, space="PSUM") as pspool,
            tc.tile_pool(name="texp", bufs=4) as tpool,
            tc.tile_pool(name="tmul", bufs=3) as mpool,
            tc.tile_pool(name="lnp", bufs=3) as lnpool,
        ):
            # Warm-up + gate-broadcast source data, memset on gpsimd (that
            # engine reaches its body first and is otherwise idle).
            warm_t = cpool.tile([128, 512], f16, tag="warm_t")
            nc.gpsimd.memset(warm_t[:], 0.125)
            ones1 = cpool.tile([1, 128], f16, tag="ones")
            nc.gpsimd.memset(ones1[:], 1.0)

            xs, ws = [], []
            for e in range(E):
                xs.append(
                    cpool.tile([128, KO, R], f16, tag=f"x{e}", name=f"x{e}")
                )
                ws.append(
                    cpool.tile([128, WROW], f16, tag=f"w{e}", name=f"w{e}")
                )
            gt = cpool.tile([128, E * R], f16, tag="g")
            gr = cpool.tile([8, E * R], f16, tag="gr")

            from concourse.bass import _add_dep_helper

            # scalar queue: x0 (split so the first 2 k-chunks land early)
            # and x1; unchained so the scalar engine proceeds to the ACT
            # table load and exps without blocking.
            d_x0a = nc.scalar.dma_start(xs[0][:, :2, :], xd[0, :, :2, :])
            nc.scalar.dma_start(xs[0][:, 2:, :], xd[0, :, 2:, :])
            d_x1 = nc.scalar.dma_start(xs[1][:], xd[1])
            _add_dep_helper(d_x1.ins, d_x0a.ins, sync=True, reason="stagger")
            # sync queue: gates + W stream unchained (5.8KB rows run near
            # peak), x2/x3 at the end chained so they don't dilute the
            # early stream; the sync engine is free to sit in those waits.
            nc.sync.dma_start(gr[:], gw[:, :])
            nc.sync.dma_start(ws[0][:, : WOFF[1]], wd[0, :, : WOFF[1]])
            nc.sync.dma_start(ws[0][:, WOFF[1] :], wd[0, :, WOFF[1] :])
            nc.sync.dma_start(ws[1][:], wd[1])
            d_w2 = nc.sync.dma_start(ws[2][:], wd[2])
            _add_dep_helper(d_w2.ins, d_x0a.ins, sync=True, reason="stagger")
            d_x2 = nc.sync.dma_start(xs[2][:], xd[2])
            _add_dep_helper(d_x2.ins, d_x1.ins, sync=True, reason="stagger")
            d_w3 = nc.sync.dma_start(ws[3][:], wd[3])
            _add_dep_helper(d_w3.ins, d_x1.ins, sync=True, reason="stagger")
            d_x3 = nc.sync.dma_start(xs[3][:], xd[3])
            _add_dep_helper(d_x3.ins, d_x2.ins, sync=True, reason="stagger")

            # PE warm-up chain.
            warm_ps = pspool.tile([128, 512], f32, tag="warm", bufs=1)

            def warm(n):
                for _ in range(n):
                    nc.tensor.matmul(
                        warm_ps[:, :],
                        warm_t[:, :128],
                        warm_t[:, :],
                        start=True,
                        stop=True,
                    )

            # The G rank-1s use a tiny dedicated PSUM pool: main-pool slots
            # would stall real matmul groups on the exp-recycle chain, and
            # reusing warm_ps would serialize rank-1s behind the DVE
            # copies (write-after-read).
            gpool_tiles = [
                pspool.tile([128, 512], f32, tag=f"gps{i}", bufs=1,
                            name=f"gps{i}")
                for i in range(2)
            ]

            def gen_g(e):
                ps_g = gpool_tiles[e % 2]
                nc.tensor.matmul(
                    ps_g[:, :],
                    ones1[:, :],
                    gr[0:1, e * R : (e + 1) * R],
                    start=True,
                    stop=True,
                )
                nc.vector.tensor_copy(gt[:, e * R : (e + 1) * R], ps_g[:, :])

            warm(6)
            gen_g(0)

            accs = [None] * NT
            ln_tiles = {}
            pend_ln = []

            def emit_ln(p_i, p0, plen, c0, cl):
                cs = slice(c0, c0 + cl)
                if p_i not in ln_tiles:
                    ln_tiles[p_i] = lnpool.tile(
                        [128, 512], f16, tag="ln", name="lnt"
                    )
                ln_t = ln_tiles[p_i]
                nc.scalar.activation(ln_t[:plen, cs], accs[p_i][:plen, cs], Ln)
                out_ap = out[p0 : p0 + plen].rearrange("p b c -> p (b c)")
                if p_i < NT - 1:
                    # Whole-tile store (1KB rows), alternating queues.
                    eng = nc.scalar if p_i % 2 else nc.sync
                    eng.dma_start(out_ap, ln_t[:plen, :])
                else:
                    # Final tile: store each column half as soon as its Ln
                    # lands, on different queues, so the issues and
                    # transfers overlap inside the tail.
                    eng = nc.sync if c0 == 0 else nc.scalar
                    eng.dma_start(out_ap[:, cs], ln_t[:plen, cs])

            for e in range(E):
                for p_i, (p0, plen) in enumerate(PTS):
                    last = e == E - 1 and p_i == NT - 1
                    ps = pspool.tile([128, 512], f32, tag="ps")
                    for ko in range(KO):
                        nc.tensor.matmul(
                            ps[:plen, :],
                            ws[e][:, WOFF[p_i] + ko * plen_w(p_i) :
                                  WOFF[p_i] + ko * plen_w(p_i) + plen],
                            xs[e][:, ko, :],
                            start=(ko == 0),
                            stop=(ko == KO - 1),
                        )
                    splits = [(0, 256), (256, 256)] if last else [(0, 512)]
                    for c0, cl in splits:
                        cs = slice(c0, c0 + cl)
                        te = tpool.tile([128, 512], f16, tag="te", name="te")
                        nc.scalar.activation(
                            te[:plen, cs],
                            ps[:plen, cs],
                            Exp,
                            bias=ws[e][:plen, p_i : p_i + 1],
                        )
                        if e == 0:
                            acc = cpool.tile(
                                [128, 512], f16, tag=f"acc{p_i}",
                                name=f"acc{p_i}",
                            )
                            accs[p_i] = acc
                            nc.vector.tensor_tensor(
                                acc[:plen, cs],
                                te[:plen, cs],
                                gt[:plen, e * R + c0 : e * R + c0 + cl],
                                Mult,
                            )
                        else:
                            acc = accs[p_i]
                            tm = mpool.tile(
                                [128, 512], f16, tag="tm", name="tm"
                            )
                            nc.vector.tensor_tensor(
                                tm[:plen, cs],
                                te[:plen, cs],
                                gt[:plen, e * R + c0 : e * R + c0 + cl],
                                Mult,
                            )
                            nc.vector.tensor_tensor(
                                acc[:plen, cs], acc[:plen, cs], tm[:plen, cs],
                                Add,
                            )
                        if e == E - 1:
                            # Defer this chunk's Ln one step so it doesn't
                            # sit ahead of the next group's Exp in the ACT
                            # engine's in-order stream.
                            pend_ln.append((p_i, p0, plen, c0, cl))
                            if len(pend_ln) >= 2:
                                emit_ln(*pend_ln.pop(0))
                    # Remaining G rank-1 broadcasts spliced in at early
                    # group boundaries on the PE.
                    if e == 0 and p_i < NT - 1:
                        gen_g(p_i + 1)
                    if e == 0 and p_i == NT - 1:
                        gen_g(3)
            while pend_ln:
                emit_ln(*pend_ln.pop(0))

    nc.compile()
    return nc


def _prep_inputs(inputs):
    gates = np.asarray(inputs["gates"], dtype=np.float32)
    Ws = [np.asarray(inputs[f"W{i}"], dtype=np.float32) for i in range(E)]
    bs = [np.asarray(inputs[f"b{i}"], dtype=np.float32) for i in range(E)]

    W = np.stack(Ws)  # [E, D, P]
    # wd[e, ki, :] packed row: bias cols then pt-major W chunks
    wt_halves = []
    for ip in range(PSPLIT):
        wt = np.zeros((E, 128, WROW), np.float16)
        wh = W[:, :, ip * PP : (ip + 1) * PP].astype(np.float16)  # [E,D,PP]
        for p_i, (p0, plen) in enumerate(PTS):
            # [E, KO, 128(ki), plen] -> [E, ki, KO*plen]
            blk = wh[:, :, p0 : p0 + plen].reshape(E, KO, 128, plen)
            blk = blk.transpose(0, 2, 1, 3).reshape(E, 128, KO * plen)
            wt[:, :, WOFF[p_i] : WOFF[p_i] + KO * plen] = blk
            for e in range(E):
                bt = np.zeros(128, np.float16)
                bt[:plen] = bs[e][ip * PP + p0 : ip * PP + p0 + plen].astype(
                    np.float16
                )
                wt[e, :, p_i] = bt
        wt_halves.append(wt)

    g_rows = []
    xt_groups = []
    for ib in range(BSPLIT):
        g = gates[ib * RB : (ib + 1) * RB, :]  # [RB, E]
        row = np.concatenate(
            [np.repeat(g[:, e], C) for e in range(E)]
        )  # [E*R]
        g_rows.append(
            np.ascontiguousarray(
                np.broadcast_to(row.astype(np.float16), (8, E * R))
            )
        )

        xts = []
        for e in range(E):
            xl = np.asarray(inputs[f"xs{e}"][ib * RB : (ib + 1) * RB, :, -1, :])
            x2 = xl.reshape(R, D).astype(np.float16)  # [R, D]
            # xd[e, ki, ko, r] = x[r, ko*128+ki]
            xts.append(
                np.ascontiguousarray(x2.reshape(R, KO, 128).transpose(2, 1, 0))
            )
        xt_groups.append(np.stack(xts))  # [E, 128, KO, R]

    in_maps = []
    for c in range(NCORES):
        ib, ip = divmod(c, PSPLIT)
        in_maps.append(
            {
                "xd": xt_groups[ib],
                "wd": wt_halves[ip],
                "gw": g_rows[ib],
            }
        )
    return in_maps


def _install_trace_support():
    """Dev-only plumbing for NTFF profiling under axon: provides the
    antenv.axon_hooks shim this image lacks and disables the S3 artifact
    upload. Returns True if tracing is usable."""
    try:
        import types

        import antenv

        if "antenv.axon_hooks" not in sys.modules:
            mod = types.ModuleType("antenv.axon_hooks")
            mod._hook = None

            def set_axon_ntff_profile_hook(h, _m=mod):
                _m._hook = h

            def get_axon_ntff_profile_hook(_m=mod):
                return _m._hook

            mod.set_axon_ntff_profile_hook = set_axon_ntff_profile_hook
            mod.get_axon_ntff_profile_hook = get_axon_ntff_profile_hook
            sys.modules["antenv.axon_hooks"] = mod
            antenv.axon_hooks = mod

        import antenv.axon_hooks as ah

        if ah.get_axon_ntff_profile_hook() is None:
            from trn_agent_boot.trn_boot import _ntff_profile_via_ctypes

            hook = _ntff_profile_via_ctypes("/opt/axon/libaxon_pjrt.so")
            if hook is None:
                return False
            ah.set_axon_ntff_profile_hook(hook)

        import concourse.bass_utils as bu

        bu.upload_artifacts = lambda tmpdir: f"local:{tmpdir}"
        return True
    except Exception as e:  # pragma: no cover - tracing is best-effort
        print(f"trace support unavailable: {type(e).__name__}: {e}")
        return False


def kernel(**inputs):
    global LAST_RESULT
    from concourse.bass_utils import run_bass_kernel_spmd

    if "nc" not in _CACHE:
        _CACHE["nc"] = _build_nc()
    nc = _CACHE["nc"]

    in_maps = _prep_inputs(inputs)
    trace = os.environ.get("BASS_KERNEL_TRACE", "0") == "1"
    if trace:
        trace = _install_trace_support()
    res = run_bass_kernel_spmd(
        nc, in_maps, core_ids=list(range(NCORES)), trace=trace
    )
    LAST_RESULT = res

    out = np.empty((B, P, C), np.float32)
    for c in range(NCORES):
        ib, ip = divmod(c, PSPLIT)
        # device output is p-major [PP, RB, C] fp16
        out[ib * RB : (ib + 1) * RB, ip * PP : (ip + 1) * PP, :] = (
            res.results[c]["out"].astype(np.float32).transpose(1, 0, 2)
        )
    return out
